# revision 17
# baseline (speedup 1.0000x reference)
"""Self-contained E8 lattice quantizer for Trainium2 (8 NeuronCores).

kernel(x) -> nearest-E8-point of each row of x [8388608, 8] f32, matching
the jax reference up to exact-f32-tie events (measure-zero on this input
distribution: ~1 row per 1M, rel err ~3e-4 << 2e-2 gate).

Math (coset-1 primary): f1 = round(x) (exact, magic-number), v = x - f1
(= d1, exact), a = |v|, s = sign(v).  Then for each row of 8:
  |d2| = 1/2 - a            (coset-2 residual magnitudes)
  argmax|d2| = argmin a ;  max|d2| = 1/2 - min a
  sum d2^2 = 2 - sum a + sum d1^2
  sum f2 = sum f1 - #[v<0] = W + S/2 - 4      (W = sum f1, S = sum s)
so with M = max a, m = min a, A = sum a, per-row:
  o1 = parity(W), o2 = parity(W + S/2)
  Delta = s2^2 - s1^2 = 2 - A + 2*o2*m - o1*(1 - 2*M);  sel = Delta < 0
  T = sel ? m : M ; Gam = sel ? -o2 : o1 ; kap = sel ? 1/2 : 0
  y = f1 + s*(kap + Gam*[a == T])
y is exact half-integers -> stored bf16 (exact), halving the store DMA.

Engines: ACT does the rounding pair + Abs + Sign; Pool (gpsimd) does
subtract and the group-sum trees; DVE does max/min reduces, the onehot
compare, the int-parity bit ops, and the bf16 tail.  The per-row scalar
chain is batched across a PAIR of tiles ([128,256] ops, one chain per
pair) to keep its latency off the critical path.

Sharding: rows split evenly across 8 cores (data parallel, no comms).
"""
import numpy as np
import concourse.bass as bass
import concourse.mybir as mybir
from concourse.tile import TileContext
from concourse.bass_utils import run_bass_kernel_spmd

AL = mybir.AluOpType
AF = mybir.ActivationFunctionType
F32 = mybir.dt.float32
BF16 = mybir.dt.bfloat16
U8 = mybir.dt.uint8
I32 = mybir.dt.int32
MAGIC = float(np.float32(12582912.0))  # 1.5 * 2^23

N_ROWS_FULL = 8388608
DIM = 8
NCORES = 8
ROWS = N_ROWS_FULL // NCORES
F = 1024  # free-dim elems per partition per tile
R = F // 8


def _split_multiwaits(nc):
    """This walrus build rejects >1 sem wait per instruction: hoist extras
    onto standalone nops inserted immediately before."""
    n = 0
    for f in nc.m.functions:
        for bb in f.blocks:
            newlist = []
            for ins in bb.instructions:
                si = getattr(ins, "sync_info", None)
                if si is not None and si.on_wait is not None and len(si.on_wait) > 1:
                    waits = list(si.on_wait)
                    for w in waits[:-1]:
                        nop = mybir.InstNoOp(name=f"I-mwfix-{n}", ins=[], outs=[])
                        n += 1
                        nop.engine = ins.engine
                        nop.sync_info = mybir.SyncInfo(on_wait=[w], on_update=[])
                        newlist.append(nop)
                    si.on_wait = [waits[-1]]
                newlist.append(ins)
            bb.instructions = newlist
    return n


def _g3(ap, c=8):
    return ap.rearrange("p (r c) -> p r c", c=c)


def _bc(ap_2d, c=8):
    p, r = ap_2d.shape
    return ap_2d.unsqueeze(2).broadcast_to((p, r, c))


def build_nc(rows=ROWS, f=F, num_devices=NCORES, fix_multiwaits=True,
             bufs=(2, 2, 2, 2)):
    elems = rows * DIM
    assert elems % (2 * 128 * f) == 0
    npairs = elems // (2 * 128 * f)
    r = f // 8

    nc = bass.Bass("TRN2", num_devices=num_devices, debug=False)
    x = nc.dram_tensor("x", [rows, DIM], F32, kind="ExternalInput")
    y = nc.dram_tensor("y", [rows, DIM], BF16, kind="ExternalOutput")
    xt = x[:].flatten().rearrange("(t p f) -> t p f", p=128, f=f)
    yt = y[:].flatten().rearrange("(t p f) -> t p f", p=128, f=f)

    def tree(dst_slice, src_ap, half_t, quarter_t):
        h3 = _g3(half_t[:], 4)
        nc.gpsimd.tensor_tensor(h3, src_ap[:, :, 0:4], src_ap[:, :, 4:8],
                                AL.add)
        q3 = _g3(quarter_t[:], 2)
        nc.gpsimd.tensor_tensor(q3, h3[:, :, 0:2], h3[:, :, 2:4], AL.add)
        nc.gpsimd.tensor_tensor(dst_slice.rearrange("p (r c) -> p r c", c=1),
                                q3[:, :, 0:1], q3[:, :, 1:2], AL.add)

    with TileContext(nc) as tc:
        with tc.tile_pool(name="io", bufs=bufs[0]) as io, \
             tc.tile_pool(name="wk", bufs=bufs[1]) as wk, \
             tc.tile_pool(name="tr", bufs=bufs[2]) as tr, \
             tc.tile_pool(name="sm", bufs=bufs[3]) as sm:

            for p_ in range(npairs):
                # pair-wide per-row scalar tiles [128, 2r]
                M2 = sm.tile([128, 2 * r], F32, tag="M2")
                m2_ = sm.tile([128, 2 * r], F32, tag="m2_")
                A2 = sm.tile([128, 2 * r], F32, tag="A2")
                W2 = sm.tile([128, 2 * r], F32, tag="W2")
                S2 = sm.tile([128, 2 * r], F32, tag="S2")

                fs, as_, ss = [], [], []
                for j in range(2):
                    t = 2 * p_ + j
                    xv = io.tile([128, f], F32, tag=f"xv{j}")
                    nc.sync.dma_start(xv[:], xt[t])
                    tm = wk.tile([128, f], F32, tag=f"tm{j}")
                    nc.scalar.activation(tm[:], xv[:], AF.Copy, bias=MAGIC)
                    f1 = wk.tile([128, f], BF16, tag=f"f1{j}")
                    nc.scalar.activation(f1[:], tm[:], AF.Copy, bias=-MAGIC)
                    v = wk.tile([128, f], F32, tag=f"v{j}")
                    nc.gpsimd.tensor_tensor(v[:], xv[:], f1[:], AL.subtract)
                    a = wk.tile([128, f], F32, tag=f"a{j}")
                    nc.scalar.activation(a[:], v[:], AF.Abs)
                    s = wk.tile([128, f], BF16, tag=f"s{j}")
                    nc.scalar.activation(s[:], v[:], AF.Sign)
                    fs.append(f1); as_.append(a); ss.append(s)

                    sl = slice(j * r, (j + 1) * r)
                    a3 = _g3(a[:])
                    nc.vector.tensor_reduce(M2[:, sl], a3,
                                            mybir.AxisListType.X, AL.max)
                    nc.vector.tensor_reduce(m2_[:, sl], a3,
                                            mybir.AxisListType.X, AL.min)
                    ah = tr.tile([128, f // 2], F32, tag=f"ah{j}")
                    aq = tr.tile([128, f // 4], F32, tag=f"aq{j}")
                    tree(A2[:, sl], a3, ah, aq)
                    wh = tr.tile([128, f // 2], BF16, tag=f"wh{j}")
                    wq = tr.tile([128, f // 4], BF16, tag=f"wq{j}")
                    tree(W2[:, sl], _g3(f1[:]), wh, wq)
                    sh = tr.tile([128, f // 2], BF16, tag=f"sh{j}")
                    sq = tr.tile([128, f // 4], BF16, tag=f"sq{j}")
                    tree(S2[:, sl], _g3(s[:]), sh, sq)

                # --- pair-wide scalar chain [128, 2r] ---
                Wi = sm.tile([128, 2 * r], I32, tag="Wi")
                nc.vector.tensor_scalar(Wi[:], W2[:], 1.0, None, AL.mult)
                o1i = sm.tile([128, 2 * r], I32, tag="o1i")
                nc.vector.tensor_scalar(o1i[:], Wi[:], 1, None, AL.bitwise_and)
                o1 = sm.tile([128, 2 * r], F32, tag="o1")
                nc.vector.tensor_scalar(o1[:], o1i[:], 1.0, None, AL.mult)
                ppi = sm.tile([128, 2 * r], I32, tag="ppi")
                nc.vector.scalar_tensor_tensor(ppi[:], S2[:], 0.5, W2[:],
                                               AL.mult, AL.add)
                o2i = sm.tile([128, 2 * r], I32, tag="o2i")
                nc.vector.tensor_scalar(o2i[:], ppi[:], 1, None, AL.bitwise_and)
                o2 = sm.tile([128, 2 * r], F32, tag="o2")
                nc.vector.tensor_scalar(o2[:], o2i[:], 1.0, None, AL.mult)
                # Dl = Delta - 2 = -A + 2*o2*m + o1*(2M - 1); sel = Dl < -2
                h1 = sm.tile([128, 2 * r], F32, tag="h1")
                nc.gpsimd.tensor_scalar(h1[:], M2[:], 2.0, -1.0, AL.mult, AL.add)
                mm2 = sm.tile([128, 2 * r], F32, tag="mm2")
                nc.gpsimd.tensor_scalar(mm2[:], m2_[:], 2.0, None, AL.mult)
                z1 = sm.tile([128, 2 * r], F32, tag="z1")
                nc.gpsimd.tensor_tensor(z1[:], o2[:], mm2[:], AL.mult)
                z2 = sm.tile([128, 2 * r], F32, tag="z2")
                nc.gpsimd.tensor_tensor(z2[:], o1[:], h1[:], AL.mult)
                z3 = sm.tile([128, 2 * r], F32, tag="z3")
                nc.gpsimd.tensor_tensor(z3[:], z1[:], z2[:], AL.add)
                Dl = sm.tile([128, 2 * r], F32, tag="Dl")
                nc.gpsimd.tensor_tensor(Dl[:], z3[:], A2[:], AL.subtract)
                selm = sm.tile([128, 2 * r], U8, tag="selm")
                nc.vector.tensor_scalar(selm[:], Dl[:], -2.0, None, AL.is_lt)
                self_ = sm.tile([128, 2 * r], F32, tag="self")
                nc.gpsimd.tensor_scalar(self_[:], selm[:], 1.0, None, AL.mult)
                kap = sm.tile([128, 2 * r], F32, tag="kap")
                nc.gpsimd.tensor_scalar(kap[:], selm[:], 0.5, None, AL.mult)
                T = sm.tile([128, 2 * r], F32, tag="T")
                nc.gpsimd.tensor_scalar(T[:], M2[:], 1.0, None, AL.mult)
                nc.vector.copy_predicated(T[:], selm[:], m2_[:])
                g0 = sm.tile([128, 2 * r], F32, tag="g0")
                nc.gpsimd.tensor_tensor(g0[:], o2[:], o1[:], AL.add)
                g1 = sm.tile([128, 2 * r], F32, tag="g1")
                nc.gpsimd.tensor_tensor(g1[:], g0[:], self_[:], AL.mult)
                Gm = sm.tile([128, 2 * r], F32, tag="Gm")
                nc.gpsimd.tensor_tensor(Gm[:], o1[:], g1[:], AL.subtract)

                # --- tails ---
                for j in range(2):
                    t = 2 * p_ + j
                    sl = slice(j * r, (j + 1) * r)
                    a3 = _g3(as_[j][:])
                    e = wk.tile([128, f], BF16, tag=f"e{j}")
                    nc.vector.tensor_tensor(_g3(e[:]), a3, _bc(T[:, sl]),
                                            AL.is_equal)
                    t1 = wk.tile([128, f], BF16, tag=f"t1{j}")
                    nc.gpsimd.tensor_tensor(_g3(t1[:]), _g3(e[:]),
                                            _bc(Gm[:, sl]), AL.mult)
                    G = wk.tile([128, f], BF16, tag=f"G{j}")
                    nc.gpsimd.tensor_tensor(_g3(G[:]), _g3(t1[:]),
                                            _bc(kap[:, sl]), AL.add)
                    t3 = wk.tile([128, f], BF16, tag=f"t3{j}")
                    nc.vector.tensor_tensor(t3[:], G[:], ss[j][:], AL.mult)
                    yv = io.tile([128, f], BF16, tag=f"yv{j}")
                    nc.vector.tensor_tensor(yv[:], fs[j][:], t3[:], AL.add)
                    nc.sync.dma_start(yt[t], yv[:])

    if fix_multiwaits:
        _split_multiwaits(nc)
    return nc


_NC_CACHE = {}


def _get_nc(rows, f):
    key = (rows, f)
    if key not in _NC_CACHE:
        _NC_CACHE[key] = build_nc(rows, f)
    return _NC_CACHE[key]


def kernel(x: np.ndarray, _trace=False) -> np.ndarray:
    assert x.shape == (N_ROWS_FULL, DIM), x.shape
    x = np.ascontiguousarray(np.asarray(x, dtype=np.float32))
    nc = _get_nc(ROWS, F)
    in_maps = [
        {"x": np.ascontiguousarray(x[i * ROWS:(i + 1) * ROWS])}
        for i in range(NCORES)
    ]
    res = run_bass_kernel_spmd(nc, in_maps, core_ids=list(range(NCORES)),
                               trace=_trace)
    out = np.empty_like(x)
    for i in range(NCORES):
        out[i * ROWS:(i + 1) * ROWS] = np.asarray(
            res.results[i]["y"], dtype=np.float32)
    return out
